# revision 16
# baseline (speedup 1.0000x reference)
"""MLA (multi-head latent attention) prefill kernel for 8 Trainium2 NeuronCores.

Problem: nn_MLA_25967372272133.
  B=2, S=2048, DIM=2048, H=16 heads, q_lora=768, kv_lora=512,
  nope=128, rope=64, v_dim=128, logit softcap 30, causal mask, XSA epilogue.

v2 design (vs v1 baseline):
- All matmul operands fp16: v1's float32r matmuls lowered to multi-pass
  fp32 (fp32_mode=HIGH, ~4 cyc/row); fp16 runs single-pass at 1 cyc/row
  with fast weight load, and fp16 rounding (5e-4) is far below the 2e-2
  error gate.
- De-replicated projections: host folds wq_b@wq_a and wkv_b@wkv_a per
  head, so each core's sweep computes only its 2 heads' q/k/v directly
  from x (384+256+256 rows) plus the shared rope key (64) and a sharded
  slice of the RMS sum-of-squares rows (96 q + 64 kv). The full ssq is
  then obtained with a tiny [2,2048] fp32 AllReduce per batch, and the
  rstd scaling is applied to the fp16 activations afterwards. This
  replaces v1's fully replicated 1344-row low-rank phase A.
- Softmax: scores stay [keys, queries]; tanh+exp on ACT, causal mask as
  a 0/1 fp16 multiply after exp, sums as fp16 DVE adds.
- XSA epilogue in transposed layout: per-token dot products via
  ones-vector matmuls; no y/sums transposes (v1 had ~190 PE transposes).
- wo partials written as fp16; host accumulates in fp32.

Pipeline: sweep(b0); AR(b0) | sweep(b1); scale+attn(b0); AR(b1);
scale+attn(b1). The AR latency hides under the other batch's sweep.
"""

import numpy as np

import concourse.bass as bass
import concourse.tile as tile
from concourse import bacc, mybir
from concourse.bass_utils import run_bass_kernel_spmd
from concourse.masks import make_identity

B, S, DIM = 2, 2048, 2048
H, NCORES, HPC = 16, 8, 2
QL, KVL = 768, 512
NOPE, ROPE, VD = 128, 64, 128
QKD = NOPE + ROPE
BS = B * S
SOFTCAP = 30.0
EPS = 1e-6
SCALE = QKD ** -0.5

F32 = mybir.dt.float32
F16 = mybir.dt.float16
AX = mybir.AxisListType
OP = mybir.AluOpType
AF = mybir.ActivationFunctionType

# W row map (columns of the folded weight):
#  0:384    q (h0 nope 128, h1 nope 128, qpe 128 = h0e h0o h1e h1o)
#  384:640  k nope (h0 128, h1 128)
#  640:896  vT (h0 128, h1 128)
#  896:1024 kpe (e32 o32) + ssq_kv shard (64)
#  1024:1120 ssq_q shard (96)
NW = 1120
SSQ_Q_SH = QL // NCORES    # 96
SSQ_KV_SH = KVL // NCORES  # 64

TT = 256                   # sweep token tile
NT = S // TT               # 8 tiles per batch
NQ = S // 512              # query tiles per batch


def _emit(nc, tc):
    xT = nc.dram_tensor("xT", [DIM, BS], F16, kind="ExternalInput").ap()
    wT = nc.dram_tensor("wT", [DIM, NW], F16, kind="ExternalInput").ap()
    woT = nc.dram_tensor("woT", [HPC * VD, DIM], F16, kind="ExternalInput").ap()
    cosT = nc.dram_tensor("cosT", [32, S], F16, kind="ExternalInput").ap()
    sinT = nc.dram_tensor("sinT", [32, S], F16, kind="ExternalInput").ap()
    m01 = nc.dram_tensor("m01", [128, 4 * 512], F16, kind="ExternalInput").ap()
    out_d = nc.dram_tensor("out", [BS, DIM], F16, kind="ExternalOutput").ap()

    xT_r = xT.rearrange("(c p) n -> p c n", p=128)

    # ---- global/persistent tiles ----
    glob = tc.alloc_tile_pool(name="glob", bufs=1)
    wo_sb = glob.tile([128, HPC, DIM], F16)
    nc.sync.dma_start(out=wo_sb, in_=woT.rearrange("(h p) m -> p h m", p=128))
    m01_sb = glob.tile([128, 4 * 512], F16)
    nc.sync.dma_start(out=m01_sb, in_=m01)
    ones16 = glob.tile([128, 1], F16)
    nc.gpsimd.memset(ones16, 1.0)
    identf = glob.tile([128, 128], F32)
    make_identity(nc, identf)
    ident16 = glob.tile([128, 128], F16)
    nc.vector.tensor_copy(ident16, identf)
    eps_sb = glob.tile([1, 1], F32)
    nc.vector.memset(eps_sb, EPS)

    # rows shared across batches (ring)
    rowp = tc.alloc_tile_pool(name="rows", bufs=1)
    # broadcast tiles ring
    bcp = tc.alloc_tile_pool(name="bc", bufs=1)

    # per-batch activation tiles
    actp = [tc.alloc_tile_pool(name=f"act{b}", bufs=1) for b in range(B)]
    qn = []
    qpe = []
    kn = []
    vT = []
    vnat = []
    kpe2 = []
    for b in range(B):
        qn.append(actp[b].tile([128, 3, S], F16, name=f"qn{b}"))
        qpe.append(actp[b].tile([128, S], F16, name=f"qpe{b}"))
        kn.append(actp[b].tile([128, HPC, S], F16, name=f"kn{b}"))
        vT.append(actp[b].tile([128, HPC, S], F16, name=f"vT{b}"))
        vnat.append(actp[b].tile([128, S // 128, HPC * VD], F16, name=f"vnat{b}"))
        kpe2.append(actp[b].tile([128, S], F16, name=f"kpe{b}"))

    # DRAM scratch for the ssq collectives
    dscr = tc.alloc_tile_pool(name="dscr", bufs=1, space="DRAM")
    cc_in = [dscr.tile([2, S], F32, name=f"ccin{b}") for b in range(B)]
    cc_out = [
        nc.dram_tensor(f"ccout{b}", [2, S], F32, addr_space="Shared").ap()
        for b in range(B)
    ]

    # psum pools (8 banks total)
    pswp = tc.alloc_tile_pool(name="pswp", bufs=2, space="PSUM")
    prow = tc.alloc_tile_pool(name="prow", bufs=2, space="PSUM")
    pscr = tc.alloc_tile_pool(name="pscr", bufs=2, space="PSUM")
    py = tc.alloc_tile_pool(name="py", bufs=2, space="PSUM")

    # ---- phase pools (closed when done) ----
    wpool = tc.alloc_tile_pool(name="wpool", bufs=1)
    w_sb = wpool.tile([128, 16, NW], F16)
    nc.sync.dma_start(out=w_sb, in_=wT.rearrange("(c p) m -> p c m", p=128))
    cos_sb = wpool.tile([32, S], F16)
    nc.sync.dma_start(out=cos_sb, in_=cosT)
    sin_sb = wpool.tile([32, S], F16)
    nc.sync.dma_start(out=sin_sb, in_=sinT)

    xpool = tc.alloc_tile_pool(name="xpool", bufs=2)
    swtmp = tc.alloc_tile_pool(name="swtmp", bufs=2)

    def sweep(b):
        for t in range(NT):
            pos = t * TT
            tsl = slice(pos, pos + TT)
            xs = xpool.tile([128, 16, TT], F16, tag="xs")
            nc.sync.dma_start(out=xs, in_=xT_r[:, :, b * S + pos:b * S + pos + TT])

            for m in range(9):
                rows = 96 if m == 8 else 128
                ps = pswp.tile([128, TT], F32, tag="mm")
                for k in range(16):
                    nc.tensor.matmul(
                        ps[:rows], w_sb[:, k, m * 128:m * 128 + rows], xs[:, k, :],
                        start=(k == 0), stop=(k == 15),
                    )
                if m < 2:  # q nope
                    nc.scalar.copy(qn[b][:, m, tsl], ps)
                elif m == 2:  # q rope (both heads)
                    _rope2(nc, swtmp, ps, cos_sb[:, tsl], sin_sb[:, tsl],
                           qpe[b], tsl, TT, heads=2)
                elif m < 5:  # k nope
                    nc.scalar.copy(kn[b][:, m - 3, tsl], ps)
                elif m < 7:  # vT
                    nc.scalar.copy(vT[b][:, m - 5, tsl], ps)
                elif m == 7:  # kpe rope rows 0:64 + ssq_kv rows 64:128
                    _rope2(nc, swtmp, ps, cos_sb[:, tsl], sin_sb[:, tsl],
                           kpe2[b], tsl, TT, heads=1, dup=True)
                    sqkv = swtmp.tile([64, TT], F16, tag="sqkv")
                    nc.scalar.activation(out=sqkv, in_=ps[64:128, :], func=AF.Square)
                    rps = prow.tile([128, TT], F32, tag="r")
                    nc.tensor.matmul(rps[32:33, :], ones16[0:64], sqkv,
                                     start=True, stop=True)
                    # stash for m == 8 to finish
                    sweep._rps = rps
                else:  # ssq_q
                    sqq = swtmp.tile([96, TT], F16, tag="sqq")
                    nc.scalar.activation(out=sqq, in_=ps[0:96, :], func=AF.Square)
                    rps = sweep._rps
                    nc.tensor.matmul(rps[0:1, :], ones16[0:96], sqq,
                                     start=True, stop=True)
                    srq = swtmp.tile([1, TT], F32, tag="srq")
                    nc.vector.tensor_copy(srq, rps[0:1, :])
                    nc.sync.dma_start(out=cc_in[b][0:1, tsl], in_=srq)
                    srk = swtmp.tile([1, TT], F32, tag="srk")
                    nc.vector.tensor_copy(srk, rps[32:33, :])
                    nc.sync.dma_start(out=cc_in[b][1:2, tsl], in_=srk)

    def all_reduce(b):
        nc.gpsimd.collective_compute(
            "AllReduce",
            mybir.AluOpType.add,
            replica_groups=[list(range(NCORES))],
            ins=[cc_in[b][:]],
            outs=[cc_out[b][:]],
        )

    def rstd_scale(b):
        rsq = rowp.tile([1, S], F32, tag="rsq")
        nc.sync.dma_start(out=rsq, in_=cc_out[b][0:1, :])
        rsk = rowp.tile([1, S], F32, tag="rsk")
        nc.sync.dma_start(out=rsk, in_=cc_out[b][1:2, :])
        nc.scalar.activation(out=rsq, in_=rsq, func=AF.Sqrt,
                             scale=1.0 / QL, bias=eps_sb)
        nc.scalar.activation(out=rsk, in_=rsk, func=AF.Sqrt,
                             scale=1.0 / KVL, bias=eps_sb)
        nc.vector.reciprocal(rsq, rsq)
        nc.vector.reciprocal(rsk, rsk)
        rq16 = rowp.tile([1, S], F16, tag="rq16")
        nc.vector.tensor_copy(rq16, rsq)
        rk16 = rowp.tile([1, S], F16, tag="rk16")
        nc.vector.tensor_copy(rk16, rsk)
        rq_bc = bcp.tile([128, S], F16, tag="rqbc")
        nc.gpsimd.partition_broadcast(rq_bc, rq16)
        rk_bc = bcp.tile([128, S], F16, tag="rkbc")
        nc.gpsimd.partition_broadcast(rk_bc, rk16)

        # in-place rstd scaling
        for m in range(3):
            nc.vector.tensor_tensor(out=qn[b][:, m, :], in0=qn[b][:, m, :],
                                    in1=rq_bc, op=OP.mult)
        nc.vector.tensor_tensor(out=qpe[b], in0=qpe[b], in1=rq_bc, op=OP.mult)
        for m in range(2):
            nc.vector.tensor_tensor(out=kn[b][:, m, :], in0=kn[b][:, m, :],
                                    in1=rk_bc, op=OP.mult)
            nc.vector.tensor_tensor(out=vT[b][:, m, :], in0=vT[b][:, m, :],
                                    in1=rk_bc, op=OP.mult)
        # v natural via PE transposes of the scaled vT
        for h in range(HPC):
            for c in range(S // 128):
                pt = prow.tile([128, 512], F32, tag="r", name="pt").bitcast(F16)
                nc.tensor.transpose(pt[:, 0:128], vT[b][:, h, c * 128:(c + 1) * 128],
                                    ident16)
                nc.scalar.copy(vnat[b][:, c, h * VD:(h + 1) * VD], pt[:, 0:128])

    def attention(b):
        for qt in range(NQ):
            qsl = slice(qt * 512, (qt + 1) * 512)
            nkc = 4 * qt + 4
            y2T = atmp.tile([128, HPC, 512], F16, tag="y2T")
            for h in range(HPC):
                psy = py.tile([128, 512], F32, tag="y")
                srps = prow.tile([128, 512], F32, tag="r")
                for kc in range(nkc):
                    ksl = slice(kc * 128, (kc + 1) * 128)
                    ps_s = pscr.tile([128, 512], F32, tag="s")
                    nc.tensor.matmul(
                        ps_s, kn[b][:, h, ksl], qn[b][:, h, qsl],
                        start=True, stop=False, skip_group_check=True,
                    )
                    hs = slice(h * 64, h * 64 + 64)
                    nc.tensor.matmul(
                        ps_s, kpe2[b][hs, ksl], qpe[b][hs, qsl],
                        start=False, stop=True, skip_group_check=True,
                    )
                    t_sb = atmp.tile([128, 512], F32, tag="t_sb")
                    nc.scalar.activation(out=t_sb, in_=ps_s, func=AF.Tanh,
                                         scale=SCALE / SOFTCAP)
                    p_sb = atmp.tile([128, 512], F16, tag="p_sb")
                    nc.scalar.activation(out=p_sb, in_=t_sb, func=AF.Exp,
                                         scale=SOFTCAP)
                    if kc >= 4 * qt:
                        d = kc - 4 * qt
                        nc.vector.tensor_tensor(
                            out=p_sb, in0=p_sb,
                            in1=m01_sb[:, d * 512:(d + 1) * 512], op=OP.mult)
                    # sums over keys accumulate in a psum row via ones-matmul
                    nc.tensor.matmul(srps[0:1, :], ones16, p_sb,
                                     start=(kc == 0), stop=(kc == nkc - 1),
                                     skip_group_check=True)
                    nc.tensor.matmul(
                        psy, vnat[b][:, kc, h * VD:(h + 1) * VD], p_sb,
                        start=(kc == 0), stop=(kc == nkc - 1),
                        skip_group_check=True,
                    )
                # ---- XSA epilogue (transposed layout) ----
                vTh = vT[b][:, h, qsl]
                prod = atmp.tile([128, 512], F16, tag="prod")
                nc.vector.tensor_tensor(out=prod, in0=psy, in1=vTh, op=OP.mult)
                vsq = atmp.tile([128, 512], F16, tag="vsq")
                nc.vector.tensor_tensor(out=vsq, in0=vTh, in1=vTh, op=OP.mult)
                rps = prow.tile([128, 512], F32, tag="r")
                nc.tensor.matmul(rps[0:1, :], ones16, prod, start=True, stop=True)
                nc.tensor.matmul(rps[32:33, :], ones16, vsq, start=True, stop=True)
                rs = artmp.tile([1, 512], F32, tag="rs")
                nc.vector.reciprocal(rs, srps[0:1, :])
                rn = artmp.tile([1, 512], F32, tag="rn")
                nc.vector.reciprocal(rn, rps[32:33, :])
                # crs = -dot * rn * rs ; applied to vT.  ys = rs applied to y.
                crs = artmp.tile([1, 512], F32, tag="crs")
                nc.vector.scalar_tensor_tensor(
                    out=crs, in0=rps[0:1, :], scalar=-1.0, in1=rn,
                    op0=OP.mult, op1=OP.mult)
                nc.vector.tensor_tensor(out=crs, in0=crs, in1=rs, op=OP.mult)
                rs_bc = artmp.tile([128, 512], F32, tag="rsbc")
                nc.gpsimd.partition_broadcast(rs_bc, rs)
                crs_bc = artmp.tile([128, 512], F32, tag="crsbc")
                nc.gpsimd.partition_broadcast(crs_bc, crs)
                # keep ya/yb fp32: y2 = ya + yb cancels near-completely at
                # early positions (XSA removes the self-value projection), so
                # rounding the addends to fp16 first would leave an absolute
                # 5e-4*|v| error on a ~0 result
                ya = atmp.tile([128, 512], F32, tag="ya")
                nc.vector.tensor_tensor(out=ya, in0=psy, in1=rs_bc, op=OP.mult)
                yb = atmp.tile([128, 512], F32, tag="yb")
                nc.vector.tensor_tensor(out=yb, in0=vTh, in1=crs_bc, op=OP.mult)
                nc.vector.tensor_tensor(out=y2T[:, h, :], in0=ya, in1=yb, op=OP.add)
            # ---- wo for this query tile ----
            for tc4 in range(4):
                o_sb = atmp.tile([128, DIM], F16, tag="o_sb")
                for oc in range(4):
                    osl = slice(oc * 512, (oc + 1) * 512)
                    po = pscr.tile([128, 512], F32, tag="s")
                    for h in range(HPC):
                        nc.tensor.matmul(
                            po, y2T[:, h, tc4 * 128:(tc4 + 1) * 128],
                            wo_sb[:, h, osl],
                            start=(h == 0), stop=(h == 1),
                        )
                    nc.scalar.copy(o_sb[:, osl], po)
                nc.sync.dma_start(
                    out=out_d[b * S + qt * 512 + tc4 * 128:
                              b * S + qt * 512 + (tc4 + 1) * 128, :],
                    in_=o_sb,
                )

    # ---- pipeline ----
    sweep(0)
    all_reduce(0)
    sweep(1)
    swtmp.release()
    xpool.release()
    wpool.release()
    atmp = tc.alloc_tile_pool(name="atmp", bufs=2)
    artmp = tc.alloc_tile_pool(name="artmp", bufs=2)
    rstd_scale(0)
    attention(0)
    all_reduce(1)
    rstd_scale(1)
    attention(1)

    # release everything in reverse stack order
    artmp.release()
    atmp.release()
    py.release()
    pscr.release()
    prow.release()
    pswp.release()
    dscr.release()
    for b in range(B - 1, -1, -1):
        actp[b].release()
    bcp.release()
    rowp.release()
    glob.release()


def _rope2(nc, pool, ps, c, s, dst, tsl, tt, heads, dup=False):
    """Rope on psum rows: per head h rows [h*64, h*64+32) even, +32 odd.
    Writes [e';o'] into dst rows h*64.. ; if dup, also duplicates into
    rows 64..128 (for the kpe stationary trick)."""
    for h in range(heads):
        xe = ps[h * 64:h * 64 + 32, :]
        xo = ps[h * 64 + 32:h * 64 + 64, :]
        t1 = pool.tile([32, tt], F16, tag="rt1")
        t2 = pool.tile([32, tt], F16, tag="rt2")
        nc.vector.tensor_tensor(out=t1, in0=xe, in1=c, op=OP.mult)
        nc.vector.tensor_tensor(out=t2, in0=xo, in1=s, op=OP.mult)
        nc.vector.tensor_tensor(out=dst[h * 64:h * 64 + 32, tsl], in0=t1, in1=t2,
                                op=OP.subtract)
        t3 = pool.tile([32, tt], F16, tag="rt3")
        t4 = pool.tile([32, tt], F16, tag="rt4")
        nc.vector.tensor_tensor(out=t3, in0=xe, in1=s, op=OP.mult)
        nc.vector.tensor_tensor(out=t4, in0=xo, in1=c, op=OP.mult)
        nc.vector.tensor_tensor(out=dst[h * 64 + 32:h * 64 + 64, tsl], in0=t3,
                                in1=t4, op=OP.add)
        if dup:
            nc.vector.tensor_copy(dst[64:128, tsl], dst[0:64, tsl])


def _build():
    nc = bacc.Bacc("TRN2", target_bir_lowering=False, debug=False,
                   num_devices=NCORES)
    with tile.TileContext(nc) as tc:
        _emit(nc, tc)
    nc.compile()
    return nc


def _prep_inputs(inputs):
    x = np.asarray(inputs["x"], np.float32)
    wq_a = np.asarray(inputs["wq_a_w"], np.float32)
    q_norm = np.asarray(inputs["q_norm_w"], np.float32)
    wq_b = np.asarray(inputs["wq_b_w"], np.float32)
    q_gain = np.asarray(inputs["q_gain"], np.float32)
    wkv_a = np.asarray(inputs["wkv_a_w"], np.float32)
    kv_norm = np.asarray(inputs["kv_norm_w"], np.float32)
    wkv_b = np.asarray(inputs["wkv_b_w"], np.float32)
    wo = np.asarray(inputs["wo_w"], np.float32)
    cos = np.asarray(inputs["freqs_cos"], np.float32)
    sin = np.asarray(inputs["freqs_sin"], np.float32)
    mask = np.asarray(inputs["mask"], np.float32)

    xT = np.ascontiguousarray(x.reshape(BS, DIM).T).astype(np.float16)

    qb = wq_b * q_norm[None, :]
    kb = wkv_b * kv_norm[None, :]
    wkv_a_kv = wkv_a[:KVL]

    def deint(rows):  # de-interleave rope pairs: evens then odds
        return np.concatenate([rows[0::2], rows[1::2]], 0)

    kpe_rows = deint(wkv_a[KVL:KVL + ROPE])  # [64, DIM]

    per_core = []
    for c in range(NCORES):
        h0, h1 = 2 * c, 2 * c + 1
        q_nope = []
        q_pe = []
        k_rows = []
        v_rows = []
        for h in (h0, h1):
            base = h * QKD
            q_nope.append(qb[base:base + NOPE] @ wq_a)
            pe = deint(qb[base + NOPE:base + QKD]) * q_gain[h]
            q_pe.append(pe @ wq_a)
            b2 = h * (NOPE + VD)
            k_rows.append(kb[b2:b2 + NOPE] @ wkv_a_kv)
            v_rows.append(kb[b2 + NOPE:b2 + NOPE + VD] @ wkv_a_kv)
        ssq_kv = wkv_a_kv[c * SSQ_KV_SH:(c + 1) * SSQ_KV_SH]
        ssq_q = wq_a[c * SSQ_Q_SH:(c + 1) * SSQ_Q_SH]
        W = np.concatenate(
            q_nope + q_pe + k_rows + v_rows + [kpe_rows, ssq_kv, ssq_q], 0)
        assert W.shape == (NW, DIM)
        wo_c = wo[:, c * HPC * VD:(c + 1) * HPC * VD]  # [DIM, 256]
        per_core.append(dict(
            wT=np.ascontiguousarray(W.T).astype(np.float16),
            woT=np.ascontiguousarray(wo_c.T).astype(np.float16),
        ))

    cosT = np.ascontiguousarray(cos.T).astype(np.float16)
    sinT = np.ascontiguousarray(sin.T).astype(np.float16)
    # 0/1 keep-mask for the 4 diagonal key blocks of each 512-query tile
    mt = mask[:512, :512].T  # [k, q]
    keep = (mt == 0.0).astype(np.float16)
    m01 = np.ascontiguousarray(
        np.concatenate([keep[d * 128:(d + 1) * 128] for d in range(4)], 1))

    shared = dict(xT=xT, cosT=cosT, sinT=sinT, m01=m01)
    return [dict(shared, **pc) for pc in per_core]


_NC_CACHE = {}


def kernel(**inputs):
    if "nc" not in _NC_CACHE:
        _NC_CACHE["nc"] = _build()
    nc = _NC_CACHE["nc"]
    in_maps = _prep_inputs(inputs)
    res = run_bass_kernel_spmd(nc, in_maps, core_ids=list(range(NCORES)))
    out = res.results[0]["out"].astype(np.float32)
    for r in res.results[1:]:
        out += r["out"].astype(np.float32)
    return out.reshape(B, S, DIM)


# revision 42
# speedup vs baseline: 1.1603x; 1.1603x over previous
"""MLA (multi-head latent attention) prefill kernel for 8 Trainium2 NeuronCores.

Problem: nn_MLA_25967372272133.
  B=2, S=2048, DIM=2048, H=16 heads, q_lora=768, kv_lora=512,
  nope=128, rope=64, v_dim=128, logit softcap 30, causal mask, XSA epilogue.

v2 design (vs v1 baseline):
- All matmul operands fp16: v1's float32r matmuls lowered to multi-pass
  fp32 (fp32_mode=HIGH, ~4 cyc/row); fp16 runs single-pass at 1 cyc/row
  with fast weight load, and fp16 rounding (5e-4) is far below the 2e-2
  error gate.
- De-replicated projections: host folds wq_b@wq_a and wkv_b@wkv_a per
  head, so each core's sweep computes only its 2 heads' q/k/v directly
  from x (384+256+256 rows) plus the shared rope key (64) and a sharded
  slice of the RMS sum-of-squares rows (96 q + 64 kv). The full ssq is
  then obtained with a tiny [2,2048] fp32 AllReduce per batch, and the
  rstd scaling is applied to the fp16 activations afterwards. This
  replaces v1's fully replicated 1344-row low-rank phase A.
- Softmax: scores stay [keys, queries]; tanh+exp on ACT, causal mask as
  a 0/1 fp16 multiply after exp, sums as fp16 DVE adds.
- XSA epilogue in transposed layout: per-token dot products via
  ones-vector matmuls; no y/sums transposes (v1 had ~190 PE transposes).
- wo partials written as fp16; host accumulates in fp32.

Pipeline: sweep(b0); AR(b0) | sweep(b1); scale+attn(b0); AR(b1);
scale+attn(b1). The AR latency hides under the other batch's sweep.
"""

import os

import numpy as np

import concourse.bass as bass
import concourse.tile as tile
from concourse import bacc, mybir
from concourse.bass_utils import run_bass_kernel_spmd
from concourse.masks import make_identity

B, S, DIM = 2, 2048, 2048
H, NCORES, HPC = 16, 8, 2
QL, KVL = 768, 512
NOPE, ROPE, VD = 128, 64, 128
QKD = NOPE + ROPE
BS = B * S
SOFTCAP = 30.0
EPS = 1e-6
SCALE = QKD ** -0.5

F32 = mybir.dt.float32
F16 = mybir.dt.float16
AX = mybir.AxisListType
OP = mybir.AluOpType
AF = mybir.ActivationFunctionType

# W row map (columns of the folded weight):
#  0:384    q (h0 nope 128, h1 nope 128, qpe 128 = h0e h0o h1e h1o)
#  384:640  k nope (h0 128, h1 128)
#  640:896  vT (h0 128, h1 128)
#  896:1024 kpe (e32 o32) + ssq_kv shard (64)
#  1024:1120 ssq_q shard (96)
NW = 1120
SSQ_Q_SH = QL // NCORES    # 96
SSQ_KV_SH = KVL // NCORES  # 64

TT = 512                   # sweep token tile
NT = S // TT               # 4 tiles per batch
NQ = S // 512              # query tiles per batch

DEBUG = os.environ.get("KDBG", "0") == "1"


def _emit(nc, tc):
    xT = nc.dram_tensor("xT", [DIM, BS], F16, kind="ExternalInput").ap()
    wT = nc.dram_tensor("wT", [DIM, NW], F16, kind="ExternalInput").ap()
    woT = nc.dram_tensor("woT", [HPC * VD, DIM], F16, kind="ExternalInput").ap()
    cosT = nc.dram_tensor("cosT", [32, S], F16, kind="ExternalInput").ap()
    sinT = nc.dram_tensor("sinT", [32, S], F16, kind="ExternalInput").ap()
    m01 = nc.dram_tensor("m01", [128, 4 * 512], F16, kind="ExternalInput").ap()
    out_d = nc.dram_tensor("out", [BS, DIM], F16, kind="ExternalOutput").ap()

    xT_r = xT.rearrange("(c p) n -> p c n", p=128)

    # ---- global/persistent tiles ----
    glob = tc.alloc_tile_pool(name="glob", bufs=1)
    wo_sb = glob.tile([128, HPC, DIM], F16)
    nc.sync.dma_start(out=wo_sb, in_=woT.rearrange("(h p) m -> p h m", p=128))
    m01_sb = glob.tile([128, 4 * 512], F16)
    nc.sync.dma_start(out=m01_sb, in_=m01)
    ones16 = glob.tile([128, 1], F16)
    nc.gpsimd.memset(ones16, 1.0)
    identf = glob.tile([128, 128], F32)
    make_identity(nc, identf)
    ident16 = glob.tile([128, 128], F16)
    nc.vector.tensor_copy(ident16, identf)
    eps_sb = glob.tile([1, 1], F32)
    nc.vector.memset(eps_sb, EPS)

    # rows shared across batches (ring)
    rowp = tc.alloc_tile_pool(name="rows", bufs=1)
    # broadcast tiles ring
    bcp = tc.alloc_tile_pool(name="bc", bufs=1)

    # per-batch activation tiles
    actp = [tc.alloc_tile_pool(name=f"act{b}", bufs=1) for b in range(B)]
    qn = []
    qpe = []
    kn = []
    vT = []
    vnat = [None, None]
    kpe2 = []
    for b in range(B):
        qn.append(actp[b].tile([128, 3, S], F16, name=f"qn{b}"))
        qpe.append(actp[b].tile([128, S], F16, name=f"qpe{b}"))
        kn.append(actp[b].tile([128, HPC, S], F16, name=f"kn{b}"))
        vT.append(actp[b].tile([128, HPC, S], F16, name=f"vT{b}"))
        kpe2.append(actp[b].tile([128, S], F16, name=f"kpe{b}"))

    # DRAM scratch for the ssq collectives
    dscr = tc.alloc_tile_pool(name="dscr", bufs=1, space="DRAM")
    cc_in = [dscr.tile([2, S], F32, name=f"ccin{b}") for b in range(B)]
    cc_out = [
        nc.dram_tensor(f"ccout{b}", [2, S], F32, addr_space="Shared").ap()
        for b in range(B)
    ]

    # psum pools (8 banks total)
    pswp = tc.alloc_tile_pool(name="pswp", bufs=2, space="PSUM")
    prow = tc.alloc_tile_pool(name="prow", bufs=2, space="PSUM")
    pscr = tc.alloc_tile_pool(name="pscr", bufs=2, space="PSUM")
    py = tc.alloc_tile_pool(name="py", bufs=2, space="PSUM")

    # ---- phase pools (closed when done) ----
    wpool = tc.alloc_tile_pool(name="wpool", bufs=1)
    w_sb = wpool.tile([128, 16, NW], F16)
    nc.sync.dma_start(out=w_sb, in_=wT.rearrange("(c p) m -> p c m", p=128))
    cos_sb = wpool.tile([32, S], F16)
    nc.sync.dma_start(out=cos_sb, in_=cosT)
    sin_sb = wpool.tile([32, S], F16)
    nc.sync.dma_start(out=sin_sb, in_=sinT)

    xpool = tc.alloc_tile_pool(name="xpool", bufs=2)
    swtmp = tc.alloc_tile_pool(name="swtmp", bufs=2)

    def sweep(b):
        for t in range(NT):
            pos = t * TT
            tsl = slice(pos, pos + TT)
            xs = xpool.tile([128, 16, TT], F16, tag="xs")
            nc.sync.dma_start(out=xs, in_=xT_r[:, :, b * S + pos:b * S + pos + TT])

            for m in range(9):
                rows = 96 if m == 8 else 128
                ps = pswp.tile([128, TT], F32, tag="mm")
                for k in range(16):
                    nc.tensor.matmul(
                        ps[:rows], w_sb[:, k, m * 128:m * 128 + rows], xs[:, k, :],
                        start=(k == 0), stop=(k == 15),
                    )
                if m < 2:  # q nope
                    nc.scalar.copy(qn[b][:, m, tsl], ps)
                elif m == 2:  # q rope (both heads)
                    _rope2(nc, swtmp, ps, cos_sb[:, tsl], sin_sb[:, tsl],
                           qpe[b], tsl, TT, heads=2)
                elif m < 5:  # k nope
                    nc.scalar.copy(kn[b][:, m - 3, tsl], ps)
                elif m < 7:  # vT
                    nc.scalar.copy(vT[b][:, m - 5, tsl], ps)
                elif m == 7:  # kpe rope rows 0:64 + ssq_kv rows 64:128
                    _rope2(nc, swtmp, ps, cos_sb[:, tsl], sin_sb[:, tsl],
                           kpe2[b], tsl, TT, heads=1, dup=True)
                    sqkv = swtmp.tile([64, TT], F16, tag="sqkv")
                    nc.scalar.activation(out=sqkv, in_=ps[64:128, :], func=AF.Square)
                    rps = prow.tile([128, TT], F32, tag="r")
                    nc.tensor.matmul(rps[32:33, :], ones16[0:64], sqkv,
                                     start=True, stop=True)
                    # stash for m == 8 to finish
                    sweep._rps = rps
                else:  # ssq_q
                    sqq = swtmp.tile([96, TT], F16, tag="sqq")
                    nc.scalar.activation(out=sqq, in_=ps[0:96, :], func=AF.Square)
                    rps = sweep._rps
                    nc.tensor.matmul(rps[0:1, :], ones16[0:96], sqq,
                                     start=True, stop=True)
                    srq = swtmp.tile([1, TT], F32, tag="srq", bufs=1)
                    nc.vector.tensor_copy(srq, rps[0:1, :])
                    nc.sync.dma_start(out=cc_in[b][0:1, tsl], in_=srq)
                    srk = swtmp.tile([1, TT], F32, tag="srk", bufs=1)
                    nc.vector.tensor_copy(srk, rps[32:33, :])
                    nc.sync.dma_start(out=cc_in[b][1:2, tsl], in_=srk)

    def all_reduce(b):
        nc.gpsimd.collective_compute(
            "AllReduce",
            mybir.AluOpType.add,
            replica_groups=[list(range(NCORES))],
            ins=[cc_in[b][:]],
            outs=[cc_out[b][:]],
        )

    def rstd_scale(b):
        # rstd rows and their broadcasts stay fp32: an fp16 rstd would put a
        # per-token-correlated +-5e-4 scale on all of a token's q/k/v, which
        # shifts scores by |s|*1e-3 without averaging (dominant error in v2)
        rsq = rowp.tile([1, S], F32, tag="rsq")
        nc.sync.dma_start(out=rsq, in_=cc_out[b][0:1, :])
        rsk = rowp.tile([1, S], F32, tag="rsk")
        nc.sync.dma_start(out=rsk, in_=cc_out[b][1:2, :])
        nc.scalar.activation(out=rsq, in_=rsq, func=AF.Sqrt,
                             scale=1.0 / QL, bias=eps_sb)
        nc.scalar.activation(out=rsk, in_=rsk, func=AF.Sqrt,
                             scale=1.0 / KVL, bias=eps_sb)
        nc.vector.reciprocal(rsq, rsq)
        nc.vector.reciprocal(rsk, rsk)
        if DEBUG and b == 0:
            nc.sync.dma_start(out=dbg["rows"][0:1, :], in_=rsq)
            nc.sync.dma_start(out=dbg["rows"][1:2, :], in_=rsk)
        rq_bc = bcp.tile([128, S], F32, tag="bc", name="rq_bc")
        nc.gpsimd.partition_broadcast(rq_bc, rsq)

        # in-place rstd scaling (fp32 broadcast x fp16 tensor -> fp16)
        for m in range(3):
            nc.vector.tensor_tensor(out=qn[b][:, m, :], in0=qn[b][:, m, :],
                                    in1=rq_bc, op=OP.mult)
        nc.vector.tensor_tensor(out=qpe[b], in0=qpe[b], in1=rq_bc, op=OP.mult)
        rk_bc = bcp.tile([128, S], F32, tag="bc", name="rk_bc")
        nc.gpsimd.partition_broadcast(rk_bc, rsk)
        for m in range(2):
            nc.vector.tensor_tensor(out=kn[b][:, m, :], in0=kn[b][:, m, :],
                                    in1=rk_bc, op=OP.mult)
            nc.vector.tensor_tensor(out=vT[b][:, m, :], in0=vT[b][:, m, :],
                                    in1=rk_bc, op=OP.mult)
        # v natural via PE transposes of the scaled vT
        for h in range(HPC):
            for c in range(S // 128):
                pt = prow.tile([128, 512], F32, tag="r", name="pt").bitcast(F16)
                nc.tensor.transpose(pt[:, 0:128], vT[b][:, h, c * 128:(c + 1) * 128],
                                    ident16)
                nc.scalar.copy(vnat[b][:, c, h * VD:(h + 1) * VD], pt[:, 0:128])

    def attention(b):
        for qt in range(NQ):
            qsl = slice(qt * 512, (qt + 1) * 512)
            nkc = 4 * qt + 4
            y2T = atmp.tile([128, HPC, 512], F16, tag="y2T")
            for h in range(HPC):
                psy = py.tile([128, 512], F32, tag="y")
                srps = prow.tile([128, 512], F32, tag="r")
                for kc in range(nkc):
                    ksl = slice(kc * 128, (kc + 1) * 128)
                    ps_s = pscr.tile([128, 512], F32, tag="s")
                    nc.tensor.matmul(
                        ps_s, kn[b][:, h, ksl], qn[b][:, h, qsl],
                        start=True, stop=False, skip_group_check=True,
                    )
                    hs = slice(h * 64, h * 64 + 64)
                    nc.tensor.matmul(
                        ps_s, kpe2[b][hs, ksl], qpe[b][hs, qsl],
                        start=False, stop=True, skip_group_check=True,
                    )
                    t_sb = atmp.tile([128, 512], F32, tag="t_sb")
                    nc.scalar.activation(out=t_sb, in_=ps_s, func=AF.Tanh,
                                         scale=SCALE / SOFTCAP)
                    p_sb = atmp.tile([128, 512], F16, tag="p_sb")
                    nc.scalar.activation(out=p_sb, in_=t_sb, func=AF.Exp,
                                         scale=SOFTCAP)
                    if kc >= 4 * qt:
                        d = kc - 4 * qt
                        nc.vector.tensor_tensor(
                            out=p_sb, in0=p_sb,
                            in1=m01_sb[:, d * 512:(d + 1) * 512], op=OP.mult)
                    if DEBUG and b == 0 and qt == 0 and h == 0 and kc == 0:
                        dblk = atmp.tile([128, 512], F32, tag="dblk", bufs=1)
                        nc.vector.tensor_copy(dblk, t_sb)
                        nc.sync.dma_start(out=dbg["tsb"], in_=dblk)
                        dblk2 = atmp.tile([128, 512], F16, tag="dblk2", bufs=1)
                        nc.vector.tensor_copy(dblk2, p_sb)
                        nc.sync.dma_start(out=dbg["psb"], in_=dblk2)
                    # sums over keys accumulate in a psum row via ones-matmul
                    nc.tensor.matmul(srps[0:1, :], ones16, p_sb,
                                     start=(kc == 0), stop=(kc == nkc - 1),
                                     skip_group_check=True)
                    nc.tensor.matmul(
                        psy, vnat[b][:, kc, h * VD:(h + 1) * VD], p_sb,
                        start=(kc == 0), stop=(kc == nkc - 1),
                        skip_group_check=True,
                    )
                # ---- XSA epilogue (transposed layout) ----
                # dot and ssqv share srps (rows 32/64) so each (h,qt) holds a
                # single prow buffer; a second tile made the 2-deep ring
                # serialize the next head's sums behind this head's row chain
                vTh = vT[b][:, h, qsl]
                prod = atmp.tile([128, 512], F16, tag="prod")
                nc.vector.tensor_tensor(out=prod, in0=psy, in1=vTh, op=OP.mult)
                vsq = atmp.tile([128, 512], F16, tag="vsq")
                nc.vector.tensor_tensor(out=vsq, in0=vTh, in1=vTh, op=OP.mult)
                rps = prow.tile([128, 512], F32, tag="r")
                nc.tensor.matmul(rps[0:1, :], ones16, prod, start=True, stop=True)
                nc.tensor.matmul(rps[32:33, :], ones16, vsq, start=True, stop=True)
                # 1/x via ACT ln+exp: vector.reciprocal costs ~5us per row
                # (1-lane), ln/exp are ~0.75us each on the scalar engine
                rs = artmp.tile([1, 512], F32, tag="rs")
                nc.scalar.activation(out=rs, in_=srps[0:1, :], func=AF.Ln)
                nc.scalar.activation(out=rs, in_=rs, func=AF.Exp, scale=-1.0)
                rn = artmp.tile([1, 512], F32, tag="rn")
                nc.scalar.activation(out=rn, in_=rps[32:33, :], func=AF.Ln)
                nc.scalar.activation(out=rn, in_=rn, func=AF.Exp, scale=-1.0)
                # crs = -dot * rn * rs ; applied to vT.  ys = rs applied to y.
                crs = artmp.tile([1, 512], F32, tag="crs")
                nc.vector.scalar_tensor_tensor(
                    out=crs, in0=rps[0:1, :], scalar=-1.0, in1=rn,
                    op0=OP.mult, op1=OP.mult)
                nc.vector.tensor_tensor(out=crs, in0=crs, in1=rs, op=OP.mult)
                if DEBUG and b == 0 and qt == 0 and h == 0:
                    drow = artmp.tile([1, 512], F32, tag="drow", bufs=4)
                    nc.vector.tensor_copy(drow, srps[0:1, :])
                    nc.sync.dma_start(out=dbg["att"][0:1, :], in_=drow)
                    drow = artmp.tile([1, 512], F32, tag="drow", bufs=4)
                    nc.vector.tensor_copy(drow, rps[0:1, :])
                    nc.sync.dma_start(out=dbg["att"][1:2, :], in_=drow)
                    drow = artmp.tile([1, 512], F32, tag="drow", bufs=4)
                    nc.vector.tensor_copy(drow, rps[32:33, :])
                    nc.sync.dma_start(out=dbg["att"][2:3, :], in_=drow)
                    nc.sync.dma_start(out=dbg["att"][3:4, :], in_=rs)
                    nc.sync.dma_start(out=dbg["att"][4:5, :], in_=rn)
                    nc.sync.dma_start(out=dbg["att"][5:6, :], in_=crs)
                rs_bc = artmp.tile([128, 512], F32, tag="rsbc")
                nc.gpsimd.partition_broadcast(rs_bc, rs)
                crs_bc = artmp.tile([128, 512], F32, tag="crsbc")
                nc.gpsimd.partition_broadcast(crs_bc, crs)
                # keep ya/yb fp32: y2 = ya + yb cancels near-completely at
                # early positions (XSA removes the self-value projection), so
                # rounding the addends to fp16 first would leave an absolute
                # 5e-4*|v| error on a ~0 result
                ya = atmp.tile([128, 512], F32, tag="ya")
                nc.vector.tensor_tensor(out=ya, in0=psy, in1=rs_bc, op=OP.mult)
                yb = atmp.tile([128, 512], F32, tag="yb")
                nc.vector.tensor_tensor(out=yb, in0=vTh, in1=crs_bc, op=OP.mult)
                nc.vector.tensor_tensor(out=y2T[:, h, :], in0=ya, in1=yb, op=OP.add)
            # ---- wo for this query tile ----
            for tc4 in range(4):
                o_sb = atmp.tile([128, DIM], F16, tag="o_sb")
                for oc in range(4):
                    osl = slice(oc * 512, (oc + 1) * 512)
                    po = pscr.tile([128, 512], F32, tag="s")
                    for h in range(HPC):
                        nc.tensor.matmul(
                            po, y2T[:, h, tc4 * 128:(tc4 + 1) * 128],
                            wo_sb[:, h, osl],
                            start=(h == 0), stop=(h == 1),
                        )
                    # DVE copy: ACT is the attention bottleneck (tanh+exp)
                    nc.vector.tensor_copy(o_sb[:, osl], po)
                nc.sync.dma_start(
                    out=out_d[b * S + qt * 512 + tc4 * 128:
                              b * S + qt * 512 + (tc4 + 1) * 128, :],
                    in_=o_sb,
                )

    dbg = {}
    if DEBUG:
        dbg["qn"] = nc.dram_tensor("dbg_qn", [128, 3, S], F16, kind="ExternalOutput").ap()
        dbg["qpe"] = nc.dram_tensor("dbg_qpe", [128, S], F16, kind="ExternalOutput").ap()
        dbg["kn"] = nc.dram_tensor("dbg_kn", [128, 2, S], F16, kind="ExternalOutput").ap()
        dbg["vT"] = nc.dram_tensor("dbg_vT", [128, 2, S], F16, kind="ExternalOutput").ap()
        dbg["kpe"] = nc.dram_tensor("dbg_kpe", [128, S], F16, kind="ExternalOutput").ap()
        dbg["vnat"] = nc.dram_tensor("dbg_vnat", [128, S // 128, HPC * VD], F16,
                                     kind="ExternalOutput").ap()
        dbg["rows"] = nc.dram_tensor("dbg_rows", [2, S], F32, kind="ExternalOutput").ap()
        dbg["att"] = nc.dram_tensor("dbg_att", [8, 512], F32, kind="ExternalOutput").ap()
        dbg["tsb"] = nc.dram_tensor("dbg_tsb", [128, 512], F32, kind="ExternalOutput").ap()
        dbg["psb"] = nc.dram_tensor("dbg_psb", [128, 512], F16, kind="ExternalOutput").ap()

    def dump_b0():
        if not DEBUG:
            return
        nc.sync.dma_start(out=dbg["qn"], in_=qn[0])
        nc.sync.dma_start(out=dbg["qpe"], in_=qpe[0])
        nc.sync.dma_start(out=dbg["kn"], in_=kn[0])
        nc.sync.dma_start(out=dbg["vT"], in_=vT[0])
        nc.sync.dma_start(out=dbg["kpe"], in_=kpe2[0])
        nc.sync.dma_start(out=dbg["vnat"], in_=vnat[0])

    # ---- pipeline ----
    sweep(0)
    all_reduce(0)
    sweep(1)
    swtmp.release()
    xpool.release()
    wpool.release()
    vpool = tc.alloc_tile_pool(name="vpool", bufs=1)
    for b in range(B):
        vnat[b] = vpool.tile([128, S // 128, HPC * VD], F16, name=f"vnat{b}")
    atmp = tc.alloc_tile_pool(name="atmp", bufs=2)
    artmp = tc.alloc_tile_pool(name="artmp", bufs=2)
    rstd_scale(0)
    attention(0)
    dump_b0()
    all_reduce(1)
    rstd_scale(1)
    attention(1)

    # release everything in reverse stack order
    artmp.release()
    atmp.release()
    vpool.release()
    py.release()
    pscr.release()
    prow.release()
    pswp.release()
    dscr.release()
    for b in range(B - 1, -1, -1):
        actp[b].release()
    bcp.release()
    rowp.release()
    glob.release()


def _rope2(nc, pool, ps, c, s, dst, tsl, tt, heads, dup=False):
    """Rope on psum rows: per head h rows [h*64, h*64+32) even, +32 odd.
    Writes [e';o'] into dst rows h*64.. ; if dup, also duplicates into
    rows 64..128 (for the kpe stationary trick)."""
    for h in range(heads):
        xe = ps[h * 64:h * 64 + 32, :]
        xo = ps[h * 64 + 32:h * 64 + 64, :]
        t1 = pool.tile([32, tt], F16, tag="rt1")
        t2 = pool.tile([32, tt], F16, tag="rt2")
        nc.vector.tensor_tensor(out=t1, in0=xe, in1=c, op=OP.mult)
        nc.vector.tensor_tensor(out=t2, in0=xo, in1=s, op=OP.mult)
        nc.vector.tensor_tensor(out=dst[h * 64:h * 64 + 32, tsl], in0=t1, in1=t2,
                                op=OP.subtract)
        t3 = pool.tile([32, tt], F16, tag="rt3")
        t4 = pool.tile([32, tt], F16, tag="rt4")
        nc.vector.tensor_tensor(out=t3, in0=xe, in1=s, op=OP.mult)
        nc.vector.tensor_tensor(out=t4, in0=xo, in1=c, op=OP.mult)
        nc.vector.tensor_tensor(out=dst[h * 64 + 32:h * 64 + 64, tsl], in0=t3,
                                in1=t4, op=OP.add)
        if dup:
            nc.vector.tensor_copy(dst[64:128, tsl], dst[0:64, tsl])


def _build():
    nc = bacc.Bacc("TRN2", target_bir_lowering=False, debug=False,
                   num_devices=NCORES)
    with tile.TileContext(nc) as tc:
        _emit(nc, tc)
    nc.compile()
    return nc


def _prep_inputs(inputs):
    x = np.asarray(inputs["x"], np.float32)
    wq_a = np.asarray(inputs["wq_a_w"], np.float32)
    q_norm = np.asarray(inputs["q_norm_w"], np.float32)
    wq_b = np.asarray(inputs["wq_b_w"], np.float32)
    q_gain = np.asarray(inputs["q_gain"], np.float32)
    wkv_a = np.asarray(inputs["wkv_a_w"], np.float32)
    kv_norm = np.asarray(inputs["kv_norm_w"], np.float32)
    wkv_b = np.asarray(inputs["wkv_b_w"], np.float32)
    wo = np.asarray(inputs["wo_w"], np.float32)
    cos = np.asarray(inputs["freqs_cos"], np.float32)
    sin = np.asarray(inputs["freqs_sin"], np.float32)
    mask = np.asarray(inputs["mask"], np.float32)

    xT = np.ascontiguousarray(x.reshape(BS, DIM).T).astype(np.float16)

    qb = wq_b * q_norm[None, :]
    kb = wkv_b * kv_norm[None, :]
    wkv_a_kv = wkv_a[:KVL]

    def deint(rows):  # de-interleave rope pairs: evens then odds
        return np.concatenate([rows[0::2], rows[1::2]], 0)

    kpe_rows = deint(wkv_a[KVL:KVL + ROPE])  # [64, DIM]

    per_core = []
    for c in range(NCORES):
        h0, h1 = 2 * c, 2 * c + 1
        q_nope = []
        q_pe = []
        k_rows = []
        v_rows = []
        for h in (h0, h1):
            base = h * QKD
            q_nope.append(qb[base:base + NOPE] @ wq_a)
            pe = deint(qb[base + NOPE:base + QKD]) * q_gain[h]
            q_pe.append(pe @ wq_a)
            b2 = h * (NOPE + VD)
            k_rows.append(kb[b2:b2 + NOPE] @ wkv_a_kv)
            v_rows.append(kb[b2 + NOPE:b2 + NOPE + VD] @ wkv_a_kv)
        ssq_kv = wkv_a_kv[c * SSQ_KV_SH:(c + 1) * SSQ_KV_SH]
        ssq_q = wq_a[c * SSQ_Q_SH:(c + 1) * SSQ_Q_SH]
        W = np.concatenate(
            q_nope + q_pe + k_rows + v_rows + [kpe_rows, ssq_kv, ssq_q], 0)
        assert W.shape == (NW, DIM)
        wo_c = wo[:, c * HPC * VD:(c + 1) * HPC * VD]  # [DIM, 256]
        per_core.append(dict(
            wT=np.ascontiguousarray(W.T).astype(np.float16),
            woT=np.ascontiguousarray(wo_c.T).astype(np.float16),
        ))

    cosT = np.ascontiguousarray(cos.T).astype(np.float16)
    sinT = np.ascontiguousarray(sin.T).astype(np.float16)
    # 0/1 keep-mask for the 4 diagonal key blocks of each 512-query tile
    mt = mask[:512, :512].T  # [k, q]
    keep = (mt == 0.0).astype(np.float16)
    m01 = np.ascontiguousarray(
        np.concatenate([keep[d * 128:(d + 1) * 128] for d in range(4)], 1))

    shared = dict(xT=xT, cosT=cosT, sinT=sinT, m01=m01)
    return [dict(shared, **pc) for pc in per_core]


_NC_CACHE = {}


def kernel(**inputs):
    if "nc" not in _NC_CACHE:
        _NC_CACHE["nc"] = _build()
    nc = _NC_CACHE["nc"]
    in_maps = _prep_inputs(inputs)
    res = run_bass_kernel_spmd(nc, in_maps, core_ids=list(range(NCORES)))
    out = res.results[0]["out"].astype(np.float32)
    for r in res.results[1:]:
        out += r["out"].astype(np.float32)
    return out.reshape(B, S, DIM)
